# revision 24
# baseline (speedup 1.0000x reference)
"""BitLinear (ternary-weight / int8-activation quantized linear) on 8 trn2 NeuronCores.

Math (matches the jax reference up to fp32 rounding):
    eta   = clip(max|x| along k, 1e-5)             per row
    x_q   = round(x * 127 / eta)    in [-127,127]  (round-half-even)
    gamma = clip(mean|w|, 1e-5)                    scalar
    w_q   = round(clip(w / gamma, -1, 1))          in {-1,0,1}
    out   = (x_q @ w_q^T) * (eta/127 * gamma) + bias

x_q / w_q are small integers exactly representable in bf16 and the PE
accumulates in fp32, so the bf16 matmul is EXACT.  Rounding uses the fp32
magic-number trick  rint(t) = (t + 1.5*2^23) - 1.5*2^23  (round-half-even).

Sharding: data-parallel over rows of x (16384 -> 2048 rows/core), weight+bias
replicated.

v2 design (vs v1 baseline at ~500us):
  - PE does ONLY the 1024 bf16 matmuls (no PE transposes, no bias matmuls).
  - All k-major transposes done by DMA xbar SBUF->SBUF (one instruction per
    [128,2048] bf16 tile, 3D strided dst), overlapped with matmuls.
  - w streamed with |w| reduce; first 4 n-tiles HELD in SBUF so they can be
    quantized the moment gamma is known; the other 12 re-read (pass2) on the
    gpsimd queue while matmuls on the held group already run.
  - bias added in fp32 via a partition-broadcast [128,N] tile (DVE
    scalar_tensor_tensor fused dequant+bias), not a rank-1 bf16 matmul.
  - elementwise work balanced across DVE / ACT / GpSimd.
"""

import os
from contextlib import ExitStack

import numpy as np

import concourse.bass as bass
import concourse.bacc as bacc
import concourse.mybir as mybir
import concourse.tile as tile
import concourse.bass_isa as bass_isa
from concourse.bass_utils import run_bass_kernel_spmd

P = 128
K = 2048
N = 2048
M_CORE = 2048
KT = K // P          # 16
NT = N // P          # 16
MT = M_CORE // P     # 16
NBLK = N // 512      # 4
N_CORES = 8
C_MAGIC = 12582912.0     # 1.5 * 2**23
INV_NK = 1.0 / (N * K)
N_HELD = 4               # w n-tiles held in SBUF across the gamma barrier

F32 = mybir.dt.float32
BF16 = mybir.dt.bfloat16
ALU = mybir.AluOpType
AXIS = mybir.AxisListType
ACTF = mybir.ActivationFunctionType


def _build_program():
    nc = bacc.Bacc("TRN2", target_bir_lowering=False, debug=False)

    x_d = nc.dram_tensor("x", [M_CORE, K], F32, kind="ExternalInput").ap()
    w_d = nc.dram_tensor("weight", [N, K], F32, kind="ExternalInput").ap()
    b_d = nc.dram_tensor("bias", [1, N], F32, kind="ExternalInput").ap()
    out_d = nc.dram_tensor("out", [M_CORE, N], F32, kind="ExternalOutput").ap()

    with tile.TileContext(nc) as tc, ExitStack() as ctx:
        stats = ctx.enter_context(tc.tile_pool(name="stats", bufs=1))
        bias_p = ctx.enter_context(tc.tile_pool(name="biasp", bufs=1))
        wqT_p = ctx.enter_context(tc.tile_pool(name="wqT", bufs=1))
        xqT_p = ctx.enter_context(tc.tile_pool(name="xqT", bufs=6))
        wstage = ctx.enter_context(tc.tile_pool(name="wstage", bufs=7))
        wqst = ctx.enter_context(tc.tile_pool(name="wqst", bufs=2))
        xstage = ctx.enter_context(tc.tile_pool(name="xstage", bufs=3))
        xqst = ctx.enter_context(tc.tile_pool(name="xqst", bufs=2))
        outst = ctx.enter_context(tc.tile_pool(name="outst", bufs=3))
        ps_mm = ctx.enter_context(
            tc.tile_pool(name="psmm", bufs=8, space=bass.MemorySpace.PSUM)
        )

        # ---- stats ----
        eta_c = stats.tile([P, MT], F32)
        inv_eta = stats.tile([P, MT], F32)
        qs_all = stats.tile([P, MT], F32)
        osc_all = stats.tile([P, MT], F32)
        wparts = stats.tile([P, NT + 3], F32)
        wsum = stats.tile([P, 1], F32)
        gsum = stats.tile([P, 1], F32)
        gamma = stats.tile([P, 1], F32)
        inv_g = stats.tile([P, 1], F32)

        # wqT layout [p, nt, kt, n]: each w-tile transpose writes a CONTIGUOUS
        # block (a strided xbar dst works, but contiguous is the validated
        # fast path); matmul rhs reads a 2-level strided [p, 4nt, 128] slice.
        wqT_all = wqT_p.tile([P, NT * KT * P], BF16)
        wqT_4d = wqT_all[:].rearrange("p (a t n) -> p a t n", a=NT, t=KT)

        bias_bc = bias_p.tile([P, N], F32)

        # ============ bias broadcast (bounces through an xstage buf) =======
        brow = xstage.tile([P, K], F32, tag="x", name="brow")
        nc.sync.dma_start(brow[0:1, 0:N], b_d[:, :])
        nc.gpsimd.partition_broadcast(bias_bc[:], brow[0:1, 0:N], channels=P)

        x_tiles = {}
        xqT_tiles = {}

        def x_iter(mt, eng=None):
            t = xstage.tile([P, K], F32, tag="x", name=f"x{mt}")
            (eng or nc.scalar).dma_start(t[:], x_d[mt * P:(mt + 1) * P, :])
            x_tiles[mt] = t

        def osc_op(mt):
            nc.vector.tensor_scalar(
                osc_all[:, mt:mt + 1], eta_c[:, mt:mt + 1],
                scalar1=gamma[:, :], scalar2=1.0 / 127.0,
                op0=ALU.mult, op1=ALU.mult)

        xq_tiles = {}

        def x_compute(mt, with_osc=True):
            t = x_tiles[mt]
            nc.vector.tensor_reduce(
                eta_c[:, mt:mt + 1], t[:], axis=AXIS.X, op=ALU.max,
                apply_absolute_value=True)
            nc.vector.tensor_scalar(
                eta_c[:, mt:mt + 1], eta_c[:, mt:mt + 1],
                scalar1=1e-5, scalar2=None, op0=ALU.max)
            nc.vector.reciprocal(inv_eta[:, mt:mt + 1], eta_c[:, mt:mt + 1])
            nc.vector.tensor_scalar(
                qs_all[:, mt:mt + 1], inv_eta[:, mt:mt + 1],
                scalar1=127.0, scalar2=None, op0=ALU.mult)
            if with_osc:
                osc_op(mt)
            nc.scalar.activation(
                t[:], t[:], ACTF.Copy, bias=C_MAGIC,
                scale=qs_all[:, mt:mt + 1])
            q = xqst.tile([P, K], BF16, tag="xq", name=f"xq{mt}")
            nc.vector.tensor_scalar(
                q[:], t[:], scalar1=C_MAGIC, scalar2=None, op0=ALU.subtract)
            xq_tiles[mt] = q

        def x_transpose(mt):
            # all xbar transposes share ONE queue: concurrent DMA-transposes
            # on two queues corrupt each other on HW
            xt = xqT_p.tile([P, KT * P], BF16, tag="xt", name=f"xt{mt}")
            xt3 = xt[:].rearrange("p (t m) -> p t m", t=KT)
            nc.sync.dma_start_transpose(xt3, xq_tiles[mt][:])
            xqT_tiles[mt] = xt3

        def x_chain(mt, with_osc=True):
            x_compute(mt, with_osc=with_osc)
            x_transpose(mt)

        # ============ early x (tiles 0..2): loads only =====================
        for mt in range(3):
            x_iter(mt)

        # ============ w pass 1 =============================================
        # trigger FIFOs kept pure: even tiles on sync ring, odd on gpsimd
        # (gp is safe here: DVE only runs non-contending reduces early).
        # partials: even -> ACT Abs in-place, odd -> DVE reduce.
        for i, nt in enumerate(list(range(N_HELD, NT)) + list(range(N_HELD))):
            t = wstage.tile([P, K], F32, tag="w", name=f"w{nt}")
            eng = nc.gpsimd if i % 2 == 0 else nc.scalar
            eng.dma_start(t[:], w_d[nt * P:(nt + 1) * P, :])
            if nt == N_HELD - 1:  # last-arriving tile: split the reduce
                nc.vector.tensor_reduce(
                    wparts[:, nt:nt + 1], t[:, 0:512], axis=AXIS.X, op=ALU.add,
                    apply_absolute_value=True)
                nc.vector.tensor_reduce(
                    wparts[:, NT:NT + 1], t[:, 512:1024], axis=AXIS.X,
                    op=ALU.add, apply_absolute_value=True)
                for j in range(2):
                    nc.scalar.activation(
                        t[:, 1024 + j * 512:1024 + (j + 1) * 512],
                        t[:, 1024 + j * 512:1024 + (j + 1) * 512],
                        ACTF.Abs, accum_out=wparts[:, NT + 1 + j:NT + 2 + j])
            elif i % 2 == 0:
                nc.scalar.activation(
                    t[:], t[:], ACTF.Abs, accum_out=wparts[:, nt:nt + 1])
            else:
                nc.vector.tensor_reduce(
                    wparts[:, nt:nt + 1], t[:], axis=AXIS.X, op=ALU.add,
                    apply_absolute_value=True)
            if i in (3, 5, 7):
                x_compute((i - 3) // 2, with_osc=False)

        # ============ pass-2 prefetch ======================================
        pass2_tiles = {}

        def pass2_load(nt):
            t = wstage.tile([P, K], F32, tag="w", name=f"w2_{nt}")
            nc.scalar.dma_start(t[:], w_d[nt * P:(nt + 1) * P, :])
            pass2_tiles[nt] = t

        for s in range(3, 6):
            x_iter(s, eng=nc.gpsimd)
        for nt in range(6):
            pass2_load(nt)

        # ============ gamma ================================================
        nc.vector.tensor_reduce(wsum[:], wparts[:], axis=AXIS.X, op=ALU.add)
        nc.gpsimd.partition_all_reduce(
            gsum[:], wsum[:], channels=P, reduce_op=bass_isa.ReduceOp.add)
        nc.vector.tensor_scalar(
            gamma[:], gsum[:], scalar1=INV_NK, scalar2=1e-5,
            op0=ALU.mult, op1=ALU.max)
        nc.vector.reciprocal(inv_g[:], gamma[:])
        for mt in range(3):
            osc_op(mt)
        for mt in range(3):
            x_transpose(mt)

        # ============ w quantize + transpose ===============================
        def w_quant(nt, t):
            # t = w/gamma + C (fp32 add rounds to the integer grid, RNE)
            if nt % 2 == 0:
                nc.vector.tensor_scalar(
                    t[:], t[:], scalar1=inv_g[:, :], scalar2=C_MAGIC,
                    op0=ALU.mult, op1=ALU.add)
            else:
                nc.scalar.activation(
                    t[:], t[:], ACTF.Copy, bias=C_MAGIC, scale=inv_g[:, :])
            nc.vector.tensor_scalar(
                t[:], t[:], scalar1=C_MAGIC, scalar2=1.0,
                op0=ALU.subtract, op1=ALU.min)
            q = wqst.tile([P, K], BF16, tag="wq", name=f"wq{nt}")
            nc.vector.tensor_scalar(
                q[:], t[:], scalar1=-1.0, scalar2=None, op0=ALU.max)
            nc.sync.dma_start_transpose(wqT_4d[:, nt, :, :], q[:])

        for nt in range(NT):
            w_quant(nt, pass2_tiles[nt])
            if nt + 6 < NT:
                pass2_load(nt + 6)

        # ============ x tiles 3..5 (needed by the wavefront) ===============
        for s in range(3, 6):
            x_chain(s)

        # ============ matmuls ==============================================
        def mm_group(mt, nb):
            ps = ps_mm.tile([P, 512], F32, tag="ps", name=f"ps{mt}_{nb}")
            for kt in range(KT):
                nc.tensor.matmul(
                    ps[:],
                    xqT_tiles[mt][:, kt, :],
                    wqT_4d[:, nb * 4:(nb + 1) * 4, kt, :],
                    start=(kt == 0),
                    stop=(kt == KT - 1),
                )
            o = outst.tile([P, 512], F32, tag="o", name=f"o{mt}_{nb}")
            nc.vector.scalar_tensor_tensor(
                o[:], ps[:], osc_all[:, mt:mt + 1],
                bias_bc[:, nb * 512:(nb + 1) * 512],
                op0=ALU.mult, op1=ALU.add)
            nc.sync.dma_start(
                out_d[mt * P:(mt + 1) * P, nb * 512:(nb + 1) * 512], o[:])

        # wavefront: nb-major over the first 6 m-tiles so the PE follows the
        # nb-group readiness (pass-2 quantize) without FIFO head-of-line
        for nb in range(NBLK):
            for mt in range(6):
                mm_group(mt, nb)

        for s in range(6, MT):
            x_iter(s)
            if s >= 8:
                for nb in range(NBLK):
                    mm_group(s - 2, nb)
            x_chain(s)
        for mt in range(MT - 2, MT):
            for nb in range(NBLK):
                mm_group(mt, nb)

    nc.compile()
    return nc


_NC_CACHE = None
LAST_EXEC_NS = None


def _get_nc():
    global _NC_CACHE
    if _NC_CACHE is None:
        _NC_CACHE = _build_program()
    return _NC_CACHE


def _make_in_maps(x, weight, bias):
    xf = np.ascontiguousarray(np.asarray(x, dtype=np.float32).reshape(-1, K))
    w = np.ascontiguousarray(np.asarray(weight, dtype=np.float32))
    b = np.ascontiguousarray(np.asarray(bias, dtype=np.float32).reshape(1, N))
    assert xf.shape[0] == N_CORES * M_CORE
    return [
        {
            "x": xf[c * M_CORE:(c + 1) * M_CORE],
            "weight": w,
            "bias": b,
        }
        for c in range(N_CORES)
    ]


def kernel(x, weight, bias):
    global LAST_EXEC_NS
    nc = _get_nc()
    in_maps = _make_in_maps(x, weight, bias)
    trace = bool(int(os.environ.get("BITLINEAR_TRACE", "0")))
    res = run_bass_kernel_spmd(nc, in_maps, list(range(N_CORES)), trace=trace)
    LAST_EXEC_NS = res.exec_time_ns
    out = np.concatenate([res.results[c]["out"] for c in range(N_CORES)], axis=0)
    return out.reshape(np.asarray(x).shape[:-1] + (N,)).astype(np.float32)


# revision 25
# speedup vs baseline: 1.1973x; 1.1973x over previous
"""BitLinear (ternary-weight / int8-activation quantized linear) on 8 trn2 NeuronCores.

Math (matches the jax reference up to fp32 rounding):
    eta   = clip(max|x| along k, 1e-5)             per row
    x_q   = round(x * 127 / eta)    in [-127,127]  (round-half-even)
    gamma = clip(mean|w|, 1e-5)                    scalar
    w_q   = round(clip(w / gamma, -1, 1))          in {-1,0,1}
    out   = (x_q @ w_q^T) * (eta/127 * gamma) + bias

x_q / w_q are small integers exactly representable in bf16 and the PE
accumulates in fp32, so the bf16 matmul is EXACT.  Rounding uses the fp32
magic-number trick  rint(t) = (t + 1.5*2^23) - 1.5*2^23  (round-half-even).

Sharding: data-parallel over rows of x (16384 -> 2048 rows/core), weight+bias
replicated.

v2 design (vs v1 baseline at ~500us):
  - PE does ONLY the 1024 bf16 matmuls (no PE transposes, no bias matmuls).
  - All k-major transposes done by DMA xbar SBUF->SBUF (one instruction per
    [128,2048] bf16 tile, 3D strided dst), overlapped with matmuls.
  - w streamed with |w| reduce; first 4 n-tiles HELD in SBUF so they can be
    quantized the moment gamma is known; the other 12 re-read (pass2) on the
    gpsimd queue while matmuls on the held group already run.
  - bias added in fp32 via a partition-broadcast [128,N] tile (DVE
    scalar_tensor_tensor fused dequant+bias), not a rank-1 bf16 matmul.
  - elementwise work balanced across DVE / ACT / GpSimd.
"""

import os
from contextlib import ExitStack

import numpy as np

import concourse.bass as bass
import concourse.bacc as bacc
import concourse.mybir as mybir
import concourse.tile as tile
import concourse.bass_isa as bass_isa
from concourse.bass_utils import run_bass_kernel_spmd

P = 128
K = 2048
N = 2048
M_CORE = 2048
KT = K // P          # 16
NT = N // P          # 16
MT = M_CORE // P     # 16
NBLK = N // 512      # 4
N_CORES = 8
C_MAGIC = 12582912.0     # 1.5 * 2**23
INV_NK = 1.0 / (N * K)
N_HELD = 4               # w n-tiles held in SBUF across the gamma barrier

F32 = mybir.dt.float32
BF16 = mybir.dt.bfloat16
ALU = mybir.AluOpType
AXIS = mybir.AxisListType
ACTF = mybir.ActivationFunctionType


def _build_program():
    nc = bacc.Bacc("TRN2", target_bir_lowering=False, debug=False)

    x_d = nc.dram_tensor("x", [M_CORE, K], F32, kind="ExternalInput").ap()
    w_d = nc.dram_tensor("weight", [N, K], F32, kind="ExternalInput").ap()
    b_d = nc.dram_tensor("bias", [1, N], F32, kind="ExternalInput").ap()
    out_d = nc.dram_tensor("out", [M_CORE, N], F32, kind="ExternalOutput").ap()

    with tile.TileContext(nc) as tc, ExitStack() as ctx:
        stats = ctx.enter_context(tc.tile_pool(name="stats", bufs=1))
        bias_p = ctx.enter_context(tc.tile_pool(name="biasp", bufs=1))
        wqT_p = ctx.enter_context(tc.tile_pool(name="wqT", bufs=1))
        xqT_p = ctx.enter_context(tc.tile_pool(name="xqT", bufs=6))
        wstage = ctx.enter_context(tc.tile_pool(name="wstage", bufs=7))
        wqst = ctx.enter_context(tc.tile_pool(name="wqst", bufs=2))
        xstage = ctx.enter_context(tc.tile_pool(name="xstage", bufs=3))
        xqst = ctx.enter_context(tc.tile_pool(name="xqst", bufs=2))
        outst = ctx.enter_context(tc.tile_pool(name="outst", bufs=3))
        ps_mm = ctx.enter_context(
            tc.tile_pool(name="psmm", bufs=8, space=bass.MemorySpace.PSUM)
        )

        # ---- stats ----
        eta_c = stats.tile([P, MT], F32)
        inv_eta = stats.tile([P, MT], F32)
        qs_all = stats.tile([P, MT], F32)
        osc_all = stats.tile([P, MT], F32)
        wparts = stats.tile([P, NT + 3], F32)
        wsum = stats.tile([P, 1], F32)
        gsum = stats.tile([P, 1], F32)
        g1s = stats.tile([1, 1], F32)
        ones_col = stats.tile([P, 1], F32)
        ones_row = stats.tile([1, P], F32)
        gamma = stats.tile([P, 1], F32)
        inv_g = stats.tile([P, 1], F32)

        # wqT layout [p, nt, kt, n]: each w-tile transpose writes a CONTIGUOUS
        # block (a strided xbar dst works, but contiguous is the validated
        # fast path); matmul rhs reads a 2-level strided [p, 4nt, 128] slice.
        wqT_all = wqT_p.tile([P, NT * KT * P], BF16)
        wqT_4d = wqT_all[:].rearrange("p (a t n) -> p a t n", a=NT, t=KT)

        bias_bc = bias_p.tile([P, N], F32)

        nc.vector.memset(ones_col[:], 1.0)
        nc.vector.memset(ones_row[:], 1.0)

        # ============ bias broadcast (bounces through an xstage buf) =======
        brow = xstage.tile([P, K], F32, tag="x", name="brow")
        nc.sync.dma_start(brow[0:1, 0:N], b_d[:, :])
        nc.gpsimd.partition_broadcast(bias_bc[:], brow[0:1, 0:N], channels=P)

        x_tiles = {}
        xqT_tiles = {}

        def x_iter(mt, eng=None):
            t = xstage.tile([P, K], F32, tag="x", name=f"x{mt}")
            (eng or nc.scalar).dma_start(t[:], x_d[mt * P:(mt + 1) * P, :])
            x_tiles[mt] = t

        def osc_op(mt):
            nc.vector.tensor_scalar(
                osc_all[:, mt:mt + 1], eta_c[:, mt:mt + 1],
                scalar1=gamma[:, :], scalar2=1.0 / 127.0,
                op0=ALU.mult, op1=ALU.mult)

        xq_tiles = {}

        def x_compute(mt, with_osc=True):
            t = x_tiles[mt]
            nc.vector.tensor_reduce(
                eta_c[:, mt:mt + 1], t[:], axis=AXIS.X, op=ALU.max,
                apply_absolute_value=True)
            nc.vector.tensor_scalar(
                eta_c[:, mt:mt + 1], eta_c[:, mt:mt + 1],
                scalar1=1e-5, scalar2=None, op0=ALU.max)
            nc.vector.reciprocal(inv_eta[:, mt:mt + 1], eta_c[:, mt:mt + 1])
            nc.vector.tensor_scalar(
                qs_all[:, mt:mt + 1], inv_eta[:, mt:mt + 1],
                scalar1=127.0, scalar2=None, op0=ALU.mult)
            if with_osc:
                osc_op(mt)
            nc.scalar.activation(
                t[:], t[:], ACTF.Copy, bias=C_MAGIC,
                scale=qs_all[:, mt:mt + 1])
            q = xqst.tile([P, K], BF16, tag="xq", name=f"xq{mt}")
            nc.vector.tensor_scalar(
                q[:], t[:], scalar1=C_MAGIC, scalar2=None, op0=ALU.subtract)
            xq_tiles[mt] = q

        def x_transpose(mt):
            # all xbar transposes share ONE queue: concurrent DMA-transposes
            # on two queues corrupt each other on HW
            xt = xqT_p.tile([P, KT * P], BF16, tag="xt", name=f"xt{mt}")
            xt3 = xt[:].rearrange("p (t m) -> p t m", t=KT)
            nc.sync.dma_start_transpose(xt3, xq_tiles[mt][:])
            xqT_tiles[mt] = xt3

        def x_chain(mt, with_osc=True):
            x_compute(mt, with_osc=with_osc)
            x_transpose(mt)

        # ============ early x (tiles 0..2): loads only =====================
        for mt in range(3):
            x_iter(mt)

        # ============ w pass 1 =============================================
        # trigger FIFOs kept pure: even tiles on sync ring, odd on gpsimd
        # (gp is safe here: DVE only runs non-contending reduces early).
        # partials: even -> ACT Abs in-place, odd -> DVE reduce.
        for i, nt in enumerate(list(range(N_HELD, NT)) + list(range(N_HELD))):
            t = wstage.tile([P, K], F32, tag="w", name=f"w{nt}")
            eng = nc.sync if i % 2 == 0 else nc.scalar
            eng.dma_start(t[:], w_d[nt * P:(nt + 1) * P, :])
            if nt == N_HELD - 1:  # last-arriving tile: split the reduce
                nc.vector.tensor_reduce(
                    wparts[:, nt:nt + 1], t[:, 0:512], axis=AXIS.X, op=ALU.add,
                    apply_absolute_value=True)
                nc.vector.tensor_reduce(
                    wparts[:, NT:NT + 1], t[:, 512:1024], axis=AXIS.X,
                    op=ALU.add, apply_absolute_value=True)
                for j in range(2):
                    nc.scalar.activation(
                        t[:, 1024 + j * 512:1024 + (j + 1) * 512],
                        t[:, 1024 + j * 512:1024 + (j + 1) * 512],
                        ACTF.Abs, accum_out=wparts[:, NT + 1 + j:NT + 2 + j])
            elif i % 2 == 0:
                nc.scalar.activation(
                    t[:], t[:], ACTF.Abs, accum_out=wparts[:, nt:nt + 1])
            else:
                nc.vector.tensor_reduce(
                    wparts[:, nt:nt + 1], t[:], axis=AXIS.X, op=ALU.add,
                    apply_absolute_value=True)
            if i in (3, 5, 7):
                x_compute((i - 3) // 2, with_osc=False)

        # ============ pass-2 prefetch ======================================
        pass2_tiles = {}

        def pass2_load(nt):
            t = wstage.tile([P, K], F32, tag="w", name=f"w2_{nt}")
            nc.scalar.dma_start(t[:], w_d[nt * P:(nt + 1) * P, :])
            pass2_tiles[nt] = t

        for s in range(3, 6):
            x_iter(s)
        for nt in range(6):
            pass2_load(nt)

        # ============ gamma ================================================
        nc.vector.tensor_reduce(wsum[:], wparts[:], axis=AXIS.X, op=ALU.add)
        # cross-partition sum + broadcast via two tiny fp32 PE matmuls (the
        # gpsimd partition_all_reduce gets starved by DVE port locks)
        pg1 = ps_mm.tile([1, 1], F32, tag="ps", name="pg1")
        nc.tensor.matmul(pg1[:], wsum[:], ones_col[:], start=True, stop=True)
        nc.vector.tensor_copy(g1s[:], pg1[:])
        pg2 = ps_mm.tile([P, 1], F32, tag="ps", name="pg2")
        nc.tensor.matmul(pg2[:], ones_row[:], g1s[:], start=True, stop=True)
        nc.vector.tensor_copy(gsum[:], pg2[:])
        nc.vector.tensor_scalar(
            gamma[:], gsum[:], scalar1=INV_NK, scalar2=1e-5,
            op0=ALU.mult, op1=ALU.max)
        nc.vector.reciprocal(inv_g[:], gamma[:])
        for mt in range(3):
            osc_op(mt)
        for mt in range(3):
            x_transpose(mt)

        # ============ w quantize + transpose ===============================
        def w_quant(nt, t):
            # t = w/gamma + C (fp32 add rounds to the integer grid, RNE)
            if nt % 2 == 0:
                nc.vector.tensor_scalar(
                    t[:], t[:], scalar1=inv_g[:, :], scalar2=C_MAGIC,
                    op0=ALU.mult, op1=ALU.add)
            else:
                nc.scalar.activation(
                    t[:], t[:], ACTF.Copy, bias=C_MAGIC, scale=inv_g[:, :])
            nc.vector.tensor_scalar(
                t[:], t[:], scalar1=C_MAGIC, scalar2=1.0,
                op0=ALU.subtract, op1=ALU.min)
            q = wqst.tile([P, K], BF16, tag="wq", name=f"wq{nt}")
            nc.vector.tensor_scalar(
                q[:], t[:], scalar1=-1.0, scalar2=None, op0=ALU.max)
            nc.sync.dma_start_transpose(wqT_4d[:, nt, :, :], q[:])

        for nt in range(NT):
            w_quant(nt, pass2_tiles[nt])
            if nt + 6 < NT:
                pass2_load(nt + 6)
            if nt == 3:
                # x tiles 3..5 (needed by the wavefront) right after nb0
                for s in range(3, 6):
                    x_chain(s)

        # ============ matmuls ==============================================
        def mm_group(mt, nb):
            ps = ps_mm.tile([P, 512], F32, tag="ps", name=f"ps{mt}_{nb}")
            for kt in range(KT):
                nc.tensor.matmul(
                    ps[:],
                    xqT_tiles[mt][:, kt, :],
                    wqT_4d[:, nb * 4:(nb + 1) * 4, kt, :],
                    start=(kt == 0),
                    stop=(kt == KT - 1),
                )
            o = outst.tile([P, 512], F32, tag="o", name=f"o{mt}_{nb}")
            nc.vector.scalar_tensor_tensor(
                o[:], ps[:], osc_all[:, mt:mt + 1],
                bias_bc[:, nb * 512:(nb + 1) * 512],
                op0=ALU.mult, op1=ALU.add)
            nc.sync.dma_start(
                out_d[mt * P:(mt + 1) * P, nb * 512:(nb + 1) * 512], o[:])

        # wavefront: nb-major over the first 6 m-tiles so the PE follows the
        # nb-group readiness (pass-2 quantize) without FIFO head-of-line
        for nb in range(NBLK):
            for mt in range(6):
                mm_group(mt, nb)

        for s in range(6, MT):
            x_iter(s)
            if s >= 8:
                for nb in range(NBLK):
                    mm_group(s - 2, nb)
            x_chain(s)
        for mt in range(MT - 2, MT):
            for nb in range(NBLK):
                mm_group(mt, nb)

    nc.compile()
    return nc


_NC_CACHE = None
LAST_EXEC_NS = None


def _get_nc():
    global _NC_CACHE
    if _NC_CACHE is None:
        _NC_CACHE = _build_program()
    return _NC_CACHE


def _make_in_maps(x, weight, bias):
    xf = np.ascontiguousarray(np.asarray(x, dtype=np.float32).reshape(-1, K))
    w = np.ascontiguousarray(np.asarray(weight, dtype=np.float32))
    b = np.ascontiguousarray(np.asarray(bias, dtype=np.float32).reshape(1, N))
    assert xf.shape[0] == N_CORES * M_CORE
    return [
        {
            "x": xf[c * M_CORE:(c + 1) * M_CORE],
            "weight": w,
            "bias": b,
        }
        for c in range(N_CORES)
    ]


def kernel(x, weight, bias):
    global LAST_EXEC_NS
    nc = _get_nc()
    in_maps = _make_in_maps(x, weight, bias)
    trace = bool(int(os.environ.get("BITLINEAR_TRACE", "0")))
    res = run_bass_kernel_spmd(nc, in_maps, list(range(N_CORES)), trace=trace)
    LAST_EXEC_NS = res.exec_time_ns
    out = np.concatenate([res.results[c]["out"] for c in range(N_CORES)], axis=0)
    return out.reshape(np.asarray(x).shape[:-1] + (N,)).astype(np.float32)
